# Initial kernel scaffold
#
"""Trainium2 Bass kernel v2 for nn_AttentionBlock (pre-norm transformer block).

Per batch b: x1 = x + Attn(LN1(x));  out = x1 + FC2(gelu(FC1(LN2(x1))))
16 heads, head_dim 64, causal, full softmax, S=2048, D=1024, MLP=4096.

Sharding (8 cores): core c -> batch b=c//2, member m=c%2.
- LN1+QKV+attention head-parallel: each core does its 8 heads over all
  2048 tokens.
- proj: partial (my 8 heads x full D) for ALL tokens, computed per
  512-token group right after that group's attention; a per-group bf16
  ReduceScatter over the pair hands each member its 256-token strip.
- Ownership: member m owns token strips [512g+256m, 512g+256m+256) for
  g=0..3 (1024 tokens, contiguous per group in x1 row order).
- LN2+MLP: per 512-token chunk (2 strips) on owned tokens; FC2 output is
  token-major so the residual add needs no final transpose.

Host-side folds (exact): ln1_w/b folded into qkv weights/biases; ln2_w/b
into fc1; proj bias pb folded into the residual input.

Engine split: Act = softmax exp (+ batched rsqrt + gelu; table switches
only at phase boundaries); DVE = all PSUM consumption + LN stats/apply;
Pool(gpsimd) = SBUF-only weight casts + causal tri mask; PE = matmuls +
transposes + 1/l broadcast via outer product.

MM mode: "bf16", or "fp8" = fp8e4(DoubleRow) matmuls for QKV/FC1/FC2
(weights host-prescaled by WS, unscaled at PSUM consumption).
"""
import numpy as np
import ml_dtypes
from contextlib import ExitStack

import concourse.bass as bass
import concourse.mybir as mybir
import concourse.tile as tile
from concourse import bacc

P = 128
DH = 64
f32 = mybir.dt.float32
bf16 = mybir.dt.bfloat16
fp8 = mybir.dt.float8e4
AF = mybir.ActivationFunctionType
ALU = mybir.AluOpType
DR = mybir.MatmulPerfMode.DoubleRow
EPS = 1e-5
WS = 32.0  # fp8 weight prescale


def build_nc(S=2048, D=1024, H=16, MLP=4096, num_devices=8,
             mm="bf16", gelu_mode="gelu"):
    HPC = H // 2                 # heads per core (8)
    KD = HPC * DH                # per-core qkv width (512)
    FT = D // P                  # feature tiles (8)
    KFT = KD // P                # per-core qkv feature tiles (4)
    NT = S // P                  # token tiles (16)
    NG = S // 512                # 512-token groups (4)
    MY = S // 2                  # owned tokens (1024)
    MT = MLP // P                # mlp feature tiles (32)
    use_dr = (mm == "fp8")
    mmdt = fp8 if use_dr else bf16
    invws = 1.0 / WS if use_dr else 1.0
    scale = float(DH) ** -0.5

    nc = bacc.Bacc(num_devices=num_devices)

    x_in = nc.dram_tensor("x", [S, D], f32, kind="ExternalInput")
    xres_in = nc.dram_tensor("x_res", [MY, D], f32, kind="ExternalInput")
    wq_in = nc.dram_tensor("wq", [D, KD], f32, kind="ExternalInput")
    wk_in = nc.dram_tensor("wk", [D, KD], f32, kind="ExternalInput")
    wv_in = nc.dram_tensor("wv", [D, KD], f32, kind="ExternalInput")
    bq_in = nc.dram_tensor("bq", [KD], f32, kind="ExternalInput")
    bk_in = nc.dram_tensor("bk", [KD], f32, kind="ExternalInput")
    bv_in = nc.dram_tensor("bv", [KD], f32, kind="ExternalInput")
    pw_in = nc.dram_tensor("pw", [KD, D], f32, kind="ExternalInput")
    fc1w_in = nc.dram_tensor("fc1w", [D, MLP], f32, kind="ExternalInput")
    fc1b_in = nc.dram_tensor("fc1b", [MLP], f32, kind="ExternalInput")
    fc2w_in = nc.dram_tensor("fc2w", [MLP, D], f32, kind="ExternalInput")
    fc2b_in = nc.dram_tensor("fc2b", [D], f32, kind="ExternalInput")
    tri_in = nc.dram_tensor("tri", [P, P], bf16, kind="ExternalInput")
    idb_in = nc.dram_tensor("identb", [P, P], bf16, kind="ExternalInput")
    out_dram = nc.dram_tensor("out", [MY, D], f32, kind="ExternalOutput")

    groups = [[i, i + 1] for i in range(0, num_devices, 2)]

    with tile.TileContext(nc) as tc, ExitStack() as es:
        const = es.enter_context(tc.tile_pool(name="const", bufs=1))
        dram = es.enter_context(tc.tile_pool(name="dram", bufs=2, space="DRAM"))

        # ---- constants / biases ----
        idb_sb = const.tile([P, P], bf16)
        nc.sync.dma_start(idb_sb[:], idb_in[:])
        tri_sb = const.tile([P, P], bf16)
        nc.sync.dma_start(tri_sb[:], tri_in[:])
        ones64 = const.tile([1, DH], bf16)
        nc.vector.memset(ones64[:], 1.0)
        eps_sb = const.tile([P, 1], f32)
        nc.vector.memset(eps_sb[:], EPS)
        bq_sb = const.tile([P, KFT], f32)
        nc.sync.dma_start(bq_sb[:], bq_in.rearrange("(o p) -> p o", p=P))
        bk_sb = const.tile([P, KFT], f32)
        nc.sync.dma_start(bk_sb[:], bk_in.rearrange("(o p) -> p o", p=P))
        bv_r = const.tile([P, KD], f32)
        nc.sync.dma_start(bv_r[:], bv_in[None, :].to_broadcast((P, KD)))
        fc1b_sb = const.tile([P, MT], f32)
        nc.sync.dma_start(fc1b_sb[:], fc1b_in.rearrange("(o p) -> p o", p=P))
        fc2b_r = const.tile([P, D], f32)
        nc.sync.dma_start(fc2b_r[:], fc2b_in[None, :].to_broadcast((P, D)))

        # LN1 stats storage
        mv1 = const.tile([P, NT, 2], f32)
        rstd1 = const.tile([P, NT], f32)
        nmn1 = const.tile([P, NT], f32)
        mv2 = const.tile([P, FT, 2], f32)
        rstd2 = const.tile([P, FT], f32)
        nmn2 = const.tile([P, FT], f32)

        # fc1 weights resident in fp8 mode (staged during attention span);
        # fc2 is staged right after attention pools close (space freed).
        wres = es.enter_context(tc.tile_pool(name="wres", bufs=1))
        fc1w_mm = wres.tile([P, FT, MLP], mmdt, name="fc1w_mm") if use_dr else None

        es_a = ExitStack()   # LN1/QKV-lifetime
        es_c = ExitStack()   # attention-lifetime
        es_0 = ExitStack()   # LN1-stats-lifetime

        # ---- es_a pools ----
        wstage = es_a.enter_context(tc.tile_pool(name="wstage", bufs=2))
        wqkv = es_a.enter_context(tc.tile_pool(name="wqkv", bufs=1))
        x_lnT = None

        # qkv/proj weights: one DMA per tensor (big transfers amortize the
        # per-DMA queue serialization), casts on Pool; queues spread SP/Pool.
        w_mm = {}
        for i, (name, w_in_t) in enumerate(
                [("wk", wk_in), ("wq", wq_in), ("wv", wv_in)]):
            w_r3 = w_in_t.rearrange("(ko ki) o -> ki ko o", ki=P)
            wb = wqkv.tile([P, FT, KD], mmdt, tag=f"w_{name}")
            for h in range(2):
                stg = wstage.tile([P, FT // 2, KD], f32, tag="wst")
                eng = nc.sync if (2 * i + h) % 2 == 0 else nc.gpsimd
                hs = slice(h * FT // 2, (h + 1) * FT // 2)
                eng.dma_start(stg[:], w_r3[:, hs, :])
                nc.gpsimd.tensor_copy(wb[:, hs, :], stg[:])
            w_mm[name] = wb

        # proj weights (bf16 always)
        pw_mm = wqkv.tile([P, KFT, D], bf16, tag="w_pw")
        pw_r3 = pw_in.rearrange("(ko ki) o -> ki ko o", ki=P)
        wstp_pool = es_a.enter_context(tc.tile_pool(name="wstp", bufs=1))

        def stage_pw():
            stgp = wstp_pool.tile([P, KFT, D], f32, tag="wstp")
            nc.sync.dma_start(stgp[:], pw_r3[:])
            nc.gpsimd.tensor_copy(pw_mm[:], stgp[:])

        pers_a = es_a.enter_context(tc.tile_pool(name="pers_a", bufs=1))
        x_lnT = pers_a.tile([P, FT, S], mmdt)

        # ---- es_c pools (attention) ----
        pers_c = es_c.enter_context(tc.tile_pool(name="pers_c", bufs=1))
        qT = pers_c.tile([P, KFT, S], bf16)
        kT = pers_c.tile([P, KFT, S], bf16)
        v_sb = pers_c.tile([P, NT, HPC, DH + 1], bf16)
        nc.vector.memset(v_sb[:, :, :, DH:DH + 1], 1.0)
        pt_pool = es_c.enter_context(tc.tile_pool(name="pt", bufs=6))
        att_sm = es_c.enter_context(tc.tile_pool(name="attsm", bufs=4))
        attnT_pool = es_c.enter_context(tc.tile_pool(name="attnT", bufs=2))
        ysb_pool = es_c.enter_context(tc.tile_pool(name="ysb", bufs=4))
        # PSUM: 8 banks total across these four pools
        ps_tr = es_c.enter_context(tc.tile_pool(name="ps_tr", bufs=1, space="PSUM"))
        ps_qkv = es_c.enter_context(tc.tile_pool(name="ps_qkv", bufs=2, space="PSUM"))
        ps_sc = es_c.enter_context(tc.tile_pool(name="ps_sc", bufs=3, space="PSUM"))
        ps_av = es_c.enter_context(tc.tile_pool(name="ps_av", bufs=2, space="PSUM"))

        y_part = [dram.tile([512, D], bf16, tag=f"y_part{g}",
                             name=f"y_part{g}") for g in range(NG)]
        y_red = [dram.tile([256, D], bf16, tag=f"y_red{g}",
                            name=f"y_red{g}") for g in range(NG)]

        def qk_slice(t, hh, cols):
            return t[DH * (hh % 2):DH * (hh % 2) + DH, hh // 2, cols]

        def ln1_chunk(ch):
            """LN1 stats + rsqrt + apply + transpose for one 512-token chunk.

            All rsqrts are emitted before the first exp, so the Act engine
            loads the sqrt table once and the exp table once.
            """
            for h2 in range(2):
                xt2 = x0_pool.tile([P, 2, D], f32, tag="xt2")
                eng = nc.scalar if (2 * ch + h2) % 2 == 0 else nc.sync
                r0 = ch * 512 + h2 * 256
                eng.dma_start(xt2[:],
                              x_in[r0:r0 + 256, :]
                              .rearrange("(a p) d -> p a d", p=P))
                for i in range(2):
                    tt = 4 * ch + 2 * h2 + i
                    st6 = pa.tile([P, 2, 6], f32, tag="st6")
                    nc.vector.bn_stats(st6[:, 0, :], xt2[:, i, 0:512])
                    nc.vector.bn_stats(st6[:, 1, :], xt2[:, i, 512:1024])
                    nc.vector.bn_aggr(mv1[:, tt, :], st6[:])
                sl = slice(4 * ch + 2 * h2, 4 * ch + 2 * h2 + 2)
                nc.scalar.activation(rstd1[:, sl], mv1[:, sl, 1], AF.Sqrt,
                                     bias=eps_sb[:])
                nc.vector.reciprocal(rstd1[:, sl], rstd1[:, sl])
                nc.vector.tensor_scalar_mul(nmn1[:, sl], mv1[:, sl, 0], -1.0)
                for i in range(2):
                    tt = 4 * ch + 2 * h2 + i
                    xn = x0_pool.tile([P, D], bf16, tag="xn")
                    nc.vector.tensor_scalar(xn[:], xt2[:, i, :],
                                            nmn1[:, tt:tt + 1],
                                            rstd1[:, tt:tt + 1],
                                            ALU.add, ALU.mult)
                    ps = ps_tr.tile([P, 1024], bf16, tag="tr")
                    for ft in range(FT):
                        nc.tensor.transpose(ps[:, ft * P:(ft + 1) * P],
                                            xn[:, ft * P:(ft + 1) * P],
                                            idb_sb[:])
                    nc.vector.tensor_copy(
                        x_lnT[:, :, tt * P:(tt + 1) * P],
                        ps[:].rearrange("p (a b) -> p a b", a=8))

        def qkv_chunk(ch):
            c0 = ch * 512
            # K and Q (feature-major out)
            for wb, bias, dstT in [(w_mm["wk"], bk_sb, kT),
                                   (w_mm["wq"], bq_sb, qT)]:
                for ot in range(KFT):
                    ps = ps_qkv.tile([P, 512], f32, tag="mm")
                    if use_dr:
                        for tc2 in range(2):
                            for t in range(4):
                                nc.tensor.matmul(
                                    ps[:, tc2 * 256:(tc2 + 1) * 256],
                                    wb[:, 2 * t:2 * t + 2, ot * P:(ot + 1) * P],
                                    x_lnT[:, 2 * t:2 * t + 2,
                                          c0 + tc2 * 256:c0 + (tc2 + 1) * 256],
                                    start=(t == 0), stop=(t == 3), perf_mode=DR)
                    else:
                        for ft in range(FT):
                            nc.tensor.matmul(
                                ps[:], wb[:, ft, ot * P:(ot + 1) * P],
                                x_lnT[:, ft, c0:c0 + 512],
                                start=(ft == 0), stop=(ft == FT - 1))
                    nc.vector.tensor_scalar(
                        dstT[:, ot, c0:c0 + 512], ps[:],
                        invws, bias[:, ot:ot + 1], ALU.mult, ALU.add)
            # V (token-major out)
            for tt in range(4 * ch, 4 * ch + 4):
                ps = ps_qkv.tile([P, 512], f32, tag="mm")
                if use_dr:
                    for vc in range(2):
                        for t in range(4):
                            nc.tensor.matmul(
                                ps[:, vc * 256:(vc + 1) * 256],
                                x_lnT[:, 2 * t:2 * t + 2, tt * P:(tt + 1) * P],
                                w_mm["wv"][:, 2 * t:2 * t + 2,
                                           vc * 256:(vc + 1) * 256],
                                start=(t == 0), stop=(t == 3), perf_mode=DR)
                else:
                    for ft in range(FT):
                        nc.tensor.matmul(
                            ps[:], x_lnT[:, ft, tt * P:(tt + 1) * P],
                            w_mm["wv"][:, ft, :],
                            start=(ft == 0), stop=(ft == FT - 1))
                nc.vector.scalar_tensor_tensor(
                    v_sb[:, tt, :, 0:DH],
                    ps[:].rearrange("p (h d) -> p h d", d=DH), invws,
                    bv_r[:].rearrange("p (h d) -> p h d", d=DH),
                    ALU.mult, ALU.add)

        def attention_group(g):
            def normalize(hh, av):
                # 1/l per query; broadcast over partitions via PE outer prod
                recip = att_sm.tile([1, 512], bf16, tag="recip")
                with nc.allow_low_precision(reason="softmax 1/l in bf16"):
                    nc.vector.reciprocal(recip[:], av[DH:DH + 1, :])
                ps_rp = ps_sc.tile([P, 512], f32, tag="sc")
                nc.tensor.matmul(ps_rp[0:DH, :], ones64[:], recip[:],
                                 start=True, stop=True)
                rep = att_sm.tile([DH, 512], f32, tag="rep_sb")
                nc.vector.tensor_copy(rep[:], ps_rp[0:DH, :])
                nc.vector.tensor_mul(
                    attnT_g[DH * (hh % 2):DH * (hh % 2) + DH, hh // 2, :],
                    av[0:DH, :], rep[:])

            prev = None
            for hh in range(HPC):
                av = ps_av.tile([P, 512], f32, tag="av")
                # j2 pairs: both score matmuls + exps, then both AV chains --
                # the second score matmul hides the first exp's latency.
                for p2 in range(0, 4 * g + 4, 2):
                    pts = []
                    for j2 in (p2, p2 + 1):
                        p0 = max(0, j2 - 4 * g) * P
                        ps_s = ps_sc.tile([P, 512], f32, tag="sc")
                        nc.tensor.matmul(
                            ps_s[:, p0:512],
                            qk_slice(kT, hh, slice(j2 * P, (j2 + 1) * P)),
                            qk_slice(qT, hh,
                                     slice(g * 512 + p0, (g + 1) * 512)),
                            start=True, stop=True)
                        pT = pt_pool.tile([P, 512], bf16, tag="pT")
                        nc.scalar.activation(pT[:, p0:512], ps_s[:, p0:512],
                                             AF.Exp, scale=scale)
                        if j2 >= 4 * g:
                            nc.gpsimd.tensor_mul(pT[:, p0:p0 + P],
                                                 pT[:, p0:p0 + P], tri_sb[:])
                        pts.append(pT)
                    for j2, pT in zip((p2, p2 + 1), pts):
                        if j2 == 0:
                            nc.tensor.matmul(
                                av[0:DH + 1, 0:512], v_sb[:, 0, hh, :],
                                pT[:, 0:512], start=True, stop=False)
                        else:
                            q0 = max(0, j2 - 4 * g)
                            if q0 == 0:
                                nc.tensor.matmul(
                                    av[0:DH + 1, 0:512], v_sb[:, j2, hh, :],
                                    pT[:, 0:512], start=False,
                                    stop=(j2 == 4 * g + 3))
                            else:
                                for qb in range(q0, 4):
                                    nc.tensor.matmul(
                                        av[0:DH + 1, qb * P:(qb + 1) * P],
                                        v_sb[:, j2, hh, :],
                                        pT[:, qb * P:(qb + 1) * P],
                                        start=False,
                                        stop=(j2 == 4 * g + 3 and qb == 3))
                # defer normalize one head so its DVE/PE chain overlaps the
                # next head's score loop
                if prev is not None:
                    normalize(*prev)
                prev = (hh, av)
            normalize(*prev)

        def proj_group(g):
            for tt2 in range(4):
                ysb = ysb_pool.tile([P, D], bf16, tag="ysb")
                for oc in range(2):
                    ps = ps_av.tile([P, 512], f32, tag="av")
                    for ftk in range(KFT):
                        nc.tensor.matmul(
                            ps[:],
                            attnT_g[:, ftk, tt2 * P:(tt2 + 1) * P],
                            pw_mm[:, ftk, oc * 512:(oc + 1) * 512],
                            start=(ftk == 0), stop=(ftk == KFT - 1))
                    nc.vector.tensor_copy(ysb[:, oc * 512:(oc + 1) * 512],
                                          ps[:])
                nc.sync.dma_start(y_part[g][tt2 * P:(tt2 + 1) * P, :], ysb[:])
            nc.gpsimd.collective_compute(
                "ReduceScatter", ALU.add, replica_groups=groups,
                ins=[y_part[g].opt()], outs=[y_red[g].opt()])

        # fc1 resident staging (fp8 mode), interleaved with attention
        fc2w_r = fc2w_in.rearrange("(ko ki) o -> ki ko o", ki=P)
        fc1w_r = fc1w_in.rearrange("(ko ki) o -> ki ko o", ki=P)

        def stage_fc1w(part, nparts=16):
            n = MLP // nparts
            stg = wstage.tile([P, FT, n], f32, tag="w1st")
            nc.sync.dma_start(stg[:],
                              fc1w_r[:, :, part * n:(part + 1) * n])
            nc.gpsimd.tensor_copy(fc1w_mm[:, :, part * n:(part + 1) * n],
                                  stg[:])

        # ======== interleaved pipeline ========
        # each attention group is emitted right after its own QKV chunk so
        # the Act exp pipeline starts as early as possible
        x0_pool = es_0.enter_context(tc.tile_pool(name="x0", bufs=2))
        pa = es_0.enter_context(tc.tile_pool(name="pa", bufs=3))
        ln1_chunk(0)
        ln1_chunk(1)
        qkv_chunk(0)
        stage_pw()
        attnT_g = attnT_pool.tile([P, KFT, 512], bf16, tag="attnT")
        attention_group(0)
        proj_group(0)
        ln1_chunk(2)
        ln1_chunk(3)
        es_0.close()
        for g in range(1, NG):
            qkv_chunk(g)
            attnT_g = attnT_pool.tile([P, KFT, 512], bf16, tag="attnT")
            attention_group(g)
            proj_group(g)
            if use_dr:
                for part in range(4 * g - 4, 4 * g):
                    stage_fc1w(part)
        if use_dr:
            for part in range(12, 16):
                stage_fc1w(part)

        es_c.close()
        es_a.close()

        # fc2 weights: staged into the space attention just freed; the DMA
        # overlaps the attention/proj drain.
        wres2 = es.enter_context(tc.tile_pool(name="wres2", bufs=1))
        fc2w_mm = wres2.tile([P, MT, D], mmdt)
        wstage2 = es.enter_context(tc.tile_pool(name="wstage2", bufs=2))
        for part in range(16):
            n = MT // 16
            stg = wstage2.tile([P, n, D], f32, tag="w2st")
            nc.scalar.dma_start(stg[:],
                                fc2w_r[:, part * n:(part + 1) * n, :])
            nc.gpsimd.tensor_copy(fc2w_mm[:, part * n:(part + 1) * n, :],
                                  stg[:])

        # ======== post-attention: x1/LN2/FC1/FC2 per 512-token chunk ========
        post = es.enter_context(tc.tile_pool(name="post", bufs=2))
        x1ln = es.enter_context(tc.tile_pool(name="x1ln", bufs=1))
        x1_lnT = x1ln.tile([P, FT, MY], mmdt)
        hT_pool = es.enter_context(tc.tile_pool(name="hT", bufs=1))
        w1pool = es.enter_context(tc.tile_pool(name="w1pool", bufs=4))
        out_pool = es.enter_context(tc.tile_pool(name="outp", bufs=2))
        ps_fc1 = es.enter_context(tc.tile_pool(name="ps_fc1", bufs=2, space="PSUM"))
        ps_fc2 = es.enter_context(tc.tile_pool(name="ps_fc2", bufs=2, space="PSUM"))
        ps_tr2 = es.enter_context(tc.tile_pool(name="ps_tr2", bufs=2, space="PSUM"))
        x1_dram = dram.tile([MY, D], f32, tag="x1_dram")

        # x1 = y_red + (x_res + pb)  [f32]; half-chunks of 2 tiles so the
        # apply reads x1 from the still-live SBUF pair (2-buf x1t ring) --
        # no DRAM round trip on the LN2 critical chain.
        for k in range(2):
            for h2 in range(2):
                tts = (4 * k + 2 * h2, 4 * k + 2 * h2 + 1)
                x1ts = []
                for tt in tts:
                    g, half = tt // 2, tt % 2
                    yr = post.tile([P, D], bf16, tag="yr")
                    nc.sync.dma_start(
                        yr[:], y_red[g][half * P:(half + 1) * P, :])
                    xr = post.tile([P, D], f32, tag="xr")
                    nc.gpsimd.dma_start(xr[:],
                                        xres_in[tt * P:(tt + 1) * P, :])
                    yrf = post.tile([P, D], f32, tag="yrf")
                    nc.vector.tensor_copy(yrf[:], yr[:])
                    x1t = post.tile([P, D], f32, tag="x1t")
                    nc.vector.tensor_add(x1t[:], yrf[:], xr[:])
                    nc.sync.dma_start(x1_dram[tt * P:(tt + 1) * P, :],
                                      x1t[:])
                    st6 = post.tile([P, 2, 6], f32, tag="st6b")
                    nc.vector.bn_stats(st6[:, 0, :], x1t[:, 0:512])
                    nc.vector.bn_stats(st6[:, 1, :], x1t[:, 512:1024])
                    nc.vector.bn_aggr(mv2[:, tt, :], st6[:])
                    x1ts.append(x1t)
                sl = slice(tts[0], tts[0] + 2)
                nc.scalar.activation(rstd2[:, sl], mv2[:, sl, 1], AF.Sqrt,
                                     bias=eps_sb[:])
                nc.vector.reciprocal(rstd2[:, sl], rstd2[:, sl])
                nc.vector.tensor_scalar_mul(nmn2[:, sl], mv2[:, sl, 0], -1.0)
                for tt, x1t in zip(tts, x1ts):
                    xn2 = post.tile([P, D], bf16, tag="xn2")
                    nc.vector.tensor_scalar(xn2[:], x1t[:],
                                            nmn2[:, tt:tt + 1],
                                            rstd2[:, tt:tt + 1],
                                            ALU.add, ALU.mult)
                    ps = ps_tr2.tile([P, 1024], bf16, tag="tr2")
                    for ft in range(FT):
                        nc.tensor.transpose(ps[:, ft * P:(ft + 1) * P],
                                            xn2[:, ft * P:(ft + 1) * P],
                                            idb_sb[:])
                    nc.vector.tensor_copy(
                        x1_lnT[:, :, tt * P:(tt + 1) * P],
                        ps[:].rearrange("p (a b) -> p a b", a=8))

            # FC1 + gelu -> hT (feature-major)
            c0 = k * 512
            hT = hT_pool.tile([P, MT, 512], mmdt, tag="hT")
            gelu_af = AF.Gelu if gelu_mode == "gelu" else AF.Tanh
            for ot in range(MT):
                if use_dr:
                    wsl = fc1w_mm[:, :, ot * P:(ot + 1) * P]
                else:
                    stg = w1pool.tile([P, FT, P], f32, tag="w1stg")
                    eng1 = nc.sync if ot % 2 == 0 else nc.gpsimd
                    eng1.dma_start(stg[:],
                                   fc1w_r[:, :, ot * P:(ot + 1) * P])
                    wsl = w1pool.tile([P, FT, P], bf16, tag="w1bf")
                    if ot % 2 == 0:
                        nc.gpsimd.tensor_copy(wsl[:], stg[:])
                    else:
                        nc.vector.tensor_copy(wsl[:], stg[:])
                ps = ps_fc1.tile([P, 512], f32, tag="mm1")
                if use_dr:
                    for tc2 in range(2):
                        for t in range(4):
                            nc.tensor.matmul(
                                ps[:, tc2 * 256:(tc2 + 1) * 256],
                                wsl[:, 2 * t:2 * t + 2, :],
                                x1_lnT[:, 2 * t:2 * t + 2,
                                       c0 + tc2 * 256:c0 + (tc2 + 1) * 256],
                                start=(t == 0), stop=(t == 3), perf_mode=DR)
                else:
                    for ft in range(FT):
                        nc.tensor.matmul(
                            ps[:], wsl[:, ft, :],
                            x1_lnT[:, ft, c0:c0 + 512],
                            start=(ft == 0), stop=(ft == FT - 1))
                nc.scalar.activation(hT[:, ot, :], ps[:], gelu_af,
                                     bias=fc1b_sb[:, ot:ot + 1], scale=invws)

            # FC2 (token-major out) + residual + store
            for tt in range(4 * k, 4 * k + 4):
                x1t = out_pool.tile([P, D], f32, tag="x1o")
                nc.gpsimd.dma_start(x1t[:], x1_dram[tt * P:(tt + 1) * P, :])
                ot_sb = x1t
                tloc = (tt - 4 * k) * P
                if use_dr:
                    for oc in range(4):
                        ps = ps_fc2.tile([P, 256], f32, tag="mm2")
                        for t in range(MT // 2):
                            nc.tensor.matmul(
                                ps[:], hT[:, 2 * t:2 * t + 2, tloc:tloc + P],
                                fc2w_mm[:, 2 * t:2 * t + 2,
                                        oc * 256:(oc + 1) * 256],
                                start=(t == 0), stop=(t == MT // 2 - 1),
                                perf_mode=DR)
                        nc.vector.scalar_tensor_tensor(
                            ot_sb[:, oc * 256:(oc + 1) * 256], ps[:], invws,
                            ot_sb[:, oc * 256:(oc + 1) * 256],
                            ALU.mult, ALU.add)
                else:
                    for oc in range(2):
                        ps = ps_fc2.tile([P, 512], f32, tag="mm2")
                        for kt in range(MT):
                            nc.tensor.matmul(
                                ps[:], hT[:, kt, tloc:tloc + P],
                                fc2w_mm[:, kt, oc * 512:(oc + 1) * 512],
                                start=(kt == 0), stop=(kt == MT - 1))
                        nc.vector.scalar_tensor_tensor(
                            ot_sb[:, oc * 512:(oc + 1) * 512], ps[:], invws,
                            ot_sb[:, oc * 512:(oc + 1) * 512],
                            ALU.mult, ALU.add)
                nc.vector.tensor_add(ot_sb[:], ot_sb[:], fc2b_r[:])
                nc.scalar.dma_start(out_dram[tt * P:(tt + 1) * P, :], ot_sb[:])
    return nc


# ---------------- host side ----------------

DIM = 1024
HEADS = 16
HEAD_DIM = DIM // HEADS
MLP_DIM = 4 * DIM
SEQ = 2048
BATCH = 4
N_CORES = 8

_nc_cache = {}


def _get_nc(mm="bf16", gelu_mode="gelu", num_devices=N_CORES):
    key = (mm, gelu_mode, num_devices)
    if key not in _nc_cache:
        nc = build_nc(S=SEQ, D=DIM, H=HEADS, MLP=MLP_DIM,
                      num_devices=num_devices, mm=mm, gelu_mode=gelu_mode)
        nc.compile()
        _nc_cache[key] = nc
    return _nc_cache[key]


def make_in_maps(x, ln1_w, ln1_b, qkv_w, qkv_b, proj_w, proj_b,
                 ln2_w, ln2_b, fc1_w, fc1_b, fc2_w, fc2_b,
                 mm="bf16", S=SEQ, D=DIM, H=HEADS, n_cores=N_CORES):
    DHh = D // H
    HPC = H // 2
    ws = WS if mm == "fp8" else 1.0
    f = np.float32
    ln1_w = np.asarray(ln1_w, f); ln1_b = np.asarray(ln1_b, f)
    ln2_w = np.asarray(ln2_w, f); ln2_b = np.asarray(ln2_b, f)
    qkv_w = np.asarray(qkv_w, f); qkv_b = np.asarray(qkv_b, f)
    proj_w = np.asarray(proj_w, f); proj_b = np.asarray(proj_b, f)
    fc1_w = np.asarray(fc1_w, f); fc1_b = np.asarray(fc1_b, f)
    fc2_w = np.asarray(fc2_w, f); fc2_b = np.asarray(fc2_b, f)
    x = np.asarray(x, f)

    # fold LN affine into the downstream linear (exact):
    #   LN(x) @ W + b == xhat @ (w[:,None]*W) + (ln_b @ W + b)
    qkv_w_f = (ln1_w[:, None] * qkv_w) * ws
    qkv_b_f = ln1_b @ qkv_w + qkv_b
    fc1_w_f = (ln2_w[:, None] * fc1_w) * ws
    fc1_b_f = ln2_b @ fc1_w + fc1_b
    fc2_w_f = fc2_w * ws

    qkv_w4 = np.ascontiguousarray(qkv_w_f.reshape(D, 3, H, DHh))
    qkv_b3 = np.ascontiguousarray(qkv_b_f.reshape(3, H, DHh))
    proj_w3 = np.ascontiguousarray(proj_w.reshape(H, DHh, D))
    tri = np.triu(np.ones((P, P))).astype(ml_dtypes.bfloat16)
    identb = np.eye(P).astype(ml_dtypes.bfloat16)

    # owned strips: member m owns [512g+256m, 512g+256m+256) for g=0..3
    common = {
        "fc1w": np.ascontiguousarray(fc1_w_f),
        "fc1b": np.ascontiguousarray(fc1_b_f),
        "fc2w": np.ascontiguousarray(fc2_w_f),
        "fc2b": np.ascontiguousarray(fc2_b),
        "tri": tri, "identb": identb,
    }
    in_maps = []
    for c in range(n_cores):
        b, m = c // 2, c % 2
        heads = slice(m * HPC, (m + 1) * HPC)
        im = dict(common)
        im["x"] = np.ascontiguousarray(x[b])
        strips = [x[b, 512 * g + 256 * m: 512 * g + 256 * m + 256] + proj_b
                  for g in range(4)]
        im["x_res"] = np.ascontiguousarray(np.concatenate(strips, axis=0), f)
        im["wq"] = np.ascontiguousarray(qkv_w4[:, 0, heads].reshape(D, -1))
        im["wk"] = np.ascontiguousarray(qkv_w4[:, 1, heads].reshape(D, -1))
        im["wv"] = np.ascontiguousarray(qkv_w4[:, 2, heads].reshape(D, -1))
        im["bq"] = np.ascontiguousarray(qkv_b3[0, heads].reshape(-1))
        im["bk"] = np.ascontiguousarray(qkv_b3[1, heads].reshape(-1))
        im["bv"] = np.ascontiguousarray(qkv_b3[2, heads].reshape(-1))
        im["pw"] = np.ascontiguousarray(proj_w3[heads].reshape(-1, D))
        in_maps.append(im)
    return in_maps


def assemble_out(results, S=SEQ, D=DIM, B=BATCH):
    full = np.empty((B, S, D), dtype=np.float32)
    for c in range(len(results)):
        b, m = c // 2, c % 2
        o = results[c]["out"]
        for g in range(4):
            full[b, 512 * g + 256 * m: 512 * g + 256 * m + 256] = \
                o[256 * g: 256 * g + 256]
    return full


def kernel(x, ln1_w, ln1_b, qkv_w, qkv_b, proj_w, proj_b,
           ln2_w, ln2_b, fc1_w, fc1_b, fc2_w, fc2_b):
    import os
    from concourse.bass_utils import run_bass_kernel_spmd
    mm = os.environ.get("MM_MODE", "bf16")
    nc = _get_nc(mm=mm)
    in_maps = make_in_maps(np.asarray(x), ln1_w, ln1_b, qkv_w, qkv_b,
                           proj_w, proj_b, ln2_w, ln2_b,
                           fc1_w, fc1_b, fc2_w, fc2_b, mm=mm)
    res = run_bass_kernel_spmd(nc, in_maps, list(range(N_CORES)), trace=False)
    return assemble_out(res.results)



# revision 11
# speedup vs baseline: 1.7594x; 1.7594x over previous
"""Trainium2 Bass kernel v3 for nn_AttentionBlock (pre-norm transformer block).

Per batch b: x1 = x + Attn(LN1(x));  out = x1 + FC2(gelu(FC1(LN2(x1))))
16 heads, head_dim 64, causal, full softmax, S=2048, D=1024, MLP=4096.

Sharding (8 cores): core c -> batch b=c//2, member m=c%2.
- LN1+QKV+attention head-parallel: each core does its 8 heads over all
  2048 tokens.
- proj: partial (my 8 heads x full D) for ALL tokens, computed per
  512-token group right after that group's attention; a per-group bf16
  ReduceScatter over the pair hands each member its 256-token strip.
- Ownership: member m owns token strips [512g+256m, 512g+256m+256) for
  g=0..3 (1024 tokens, contiguous per group in x1 row order).
- LN2+MLP: per 512-token chunk (2 strips) on owned tokens; FC2 output is
  token-major so the residual add needs no final transpose.

v3 changes vs v2:
- All weights host-pretiled to the exact SBUF layout and host-cast to the
  matmul dtype (fp8e4 x WS for qkv/fc1/fc2, bf16 for proj): no on-chip
  staging or Pool casts, and every weight DMA moves >=512B contiguous
  chunks per partition.
- x for LN1 arrives bf16 (stats tolerance ~0.4%, way under fp8 noise);
  x_res stays f32 for the residual.
- fp8 DoubleRow matmuls for QKV/FC1/FC2 by default (mm="bf16" fallback).
- x1 lives in SBUF across the post phase (no x1 DRAM round trip).

Host-side folds (exact): ln1_w/b folded into qkv weights/biases; ln2_w/b
into fc1; proj bias pb folded into the residual input.

Engine split: Act = softmax exp (+ batched rsqrt + gelu); DVE = all PSUM
consumption + LN stats/apply; Pool = causal tri mask + x_res loads
(SWDGE); PE = matmuls + transposes + 1/l broadcast via outer product.
"""
import numpy as np
import ml_dtypes
from contextlib import ExitStack

import concourse.bass as bass
import concourse.mybir as mybir
import concourse.tile as tile
from concourse import bacc

P = 128
DH = 64
f32 = mybir.dt.float32
bf16 = mybir.dt.bfloat16
fp8 = mybir.dt.float8e4
AF = mybir.ActivationFunctionType
ALU = mybir.AluOpType
DR = mybir.MatmulPerfMode.DoubleRow
EPS = 1e-5
WS = 32.0  # fp8 weight prescale


def build_nc(S=2048, D=1024, H=16, MLP=4096, num_devices=8, mm="fp8"):
    HPC = H // 2                 # heads per core (8)
    KD = HPC * DH                # per-core qkv width (512)
    FT = D // P                  # feature tiles (8)
    KFT = KD // P                # per-core qkv feature tiles (4)
    NT = S // P                  # token tiles (16)
    NG = S // 512                # 512-token groups (4)
    MY = S // 2                  # owned tokens (1024)
    MT = MLP // P                # mlp feature tiles (32)
    use_dr = (mm == "fp8")
    mmdt = fp8 if use_dr else bf16
    invws = 1.0 / WS if use_dr else 1.0
    scale = float(DH) ** -0.5

    nc = bacc.Bacc(num_devices=num_devices)

    x_in = nc.dram_tensor("x", [S, D], bf16, kind="ExternalInput")
    xres_in = nc.dram_tensor("x_res", [MY, D], f32, kind="ExternalInput")
    wq_in = nc.dram_tensor("wq", [P, FT, KD], mmdt, kind="ExternalInput")
    wk_in = nc.dram_tensor("wk", [P, FT, KD], mmdt, kind="ExternalInput")
    wv_in = nc.dram_tensor("wv", [P, FT, KD], mmdt, kind="ExternalInput")
    bq_in = nc.dram_tensor("bq", [P, KFT], f32, kind="ExternalInput")
    bk_in = nc.dram_tensor("bk", [P, KFT], f32, kind="ExternalInput")
    bv_in = nc.dram_tensor("bv", [KD], f32, kind="ExternalInput")
    pw_in = nc.dram_tensor("pw", [P, KFT, D], bf16, kind="ExternalInput")
    fc1w_in = nc.dram_tensor("fc1w", [P, FT, MLP], mmdt, kind="ExternalInput")
    fc1b_in = nc.dram_tensor("fc1b", [P, MT], f32, kind="ExternalInput")
    fc2w_in = nc.dram_tensor("fc2w", [P, MT, D], mmdt, kind="ExternalInput")
    fc2b_in = nc.dram_tensor("fc2b", [D], f32, kind="ExternalInput")
    tri_in = nc.dram_tensor("tri", [P, P], bf16, kind="ExternalInput")
    idb_in = nc.dram_tensor("identb", [P, P], bf16, kind="ExternalInput")
    out_dram = nc.dram_tensor("out", [MY, D], f32, kind="ExternalOutput")

    groups = [[i, i + 1] for i in range(0, num_devices, 2)]

    with tile.TileContext(nc) as tc, ExitStack() as es:
        const = es.enter_context(tc.tile_pool(name="const", bufs=1))
        dram = es.enter_context(tc.tile_pool(name="dram", bufs=2, space="DRAM"))

        # ---- constants / biases (scalar queue; tiny) ----
        idb_sb = const.tile([P, P], bf16)
        nc.scalar.dma_start(idb_sb[:], idb_in[:])
        tri_sb = const.tile([P, P], bf16)
        nc.scalar.dma_start(tri_sb[:], tri_in[:])
        ones64 = const.tile([1, DH], bf16)
        nc.vector.memset(ones64[:], 1.0)
        eps_sb = const.tile([P, 1], f32)
        nc.vector.memset(eps_sb[:], EPS)
        bq_sb = const.tile([P, KFT], f32)
        nc.scalar.dma_start(bq_sb[:], bq_in[:])
        bk_sb = const.tile([P, KFT], f32)
        nc.scalar.dma_start(bk_sb[:], bk_in[:])
        bv_r = const.tile([P, KD], f32)
        nc.scalar.dma_start(bv_r[:], bv_in[None, :].to_broadcast((P, KD)))
        fc1b_sb = const.tile([P, MT], f32)
        fc2b_r = const.tile([P, D], f32)

        # LN stats storage
        mv1 = const.tile([P, NT, 2], f32)
        rstd1 = const.tile([P, NT], f32)
        nmn1 = const.tile([P, NT], f32)
        mv2 = const.tile([P, FT, 2], f32)
        rstd2 = const.tile([P, FT], f32)
        nmn2 = const.tile([P, FT], f32)

        # fc1 weights resident in fp8 (loaded during LN1/QKV span, used in
        # post); streamed per-2ot in the bf16 fallback (too big resident)
        if use_dr:
            wres = es.enter_context(tc.tile_pool(name="wres", bufs=1))
            fc1w_sb = wres.tile([P, FT, MLP], mmdt, name="fc1w_sb")
        else:
            fc1w_sb = None

        es_a = ExitStack()   # LN1/QKV-lifetime
        es_c = ExitStack()   # attention-lifetime
        es_0 = ExitStack()   # LN1-stats-lifetime

        # ---- es_a pools: qkv/proj weights, direct DMA (pre-tiled).
        # DMAs are emitted inside the pipeline (after ln1_chunk(0)) so the
        # first x loads aren't queued behind them.
        wqkv = es_a.enter_context(tc.tile_pool(name="wqkv", bufs=1))
        w_mm = {}
        for name in ("wk", "wq", "wv"):
            w_mm[name] = wqkv.tile([P, FT, KD], mmdt, tag=f"w_{name}",
                                   name=f"w_{name}")
        pw_mm = wqkv.tile([P, KFT, D], bf16, tag="w_pw")

        def load_qkv_weights():
            for name, w_in_t in [("wk", wk_in), ("wq", wq_in), ("wv", wv_in)]:
                nc.scalar.dma_start(w_mm[name][:], w_in_t[:])
            nc.scalar.dma_start(pw_mm[:], pw_in[:])

        pers_a = es_a.enter_context(tc.tile_pool(name="pers_a", bufs=1))
        x_lnT = pers_a.tile([P, FT, S], mmdt)

        # ---- es_c pools (attention) ----
        pers_c = es_c.enter_context(tc.tile_pool(name="pers_c", bufs=1))
        qT = pers_c.tile([P, KFT, S], bf16)
        kT = pers_c.tile([P, KFT, S], bf16)
        v_sb = pers_c.tile([P, NT, HPC, DH + 1], bf16)
        nc.vector.memset(v_sb[:, :, :, DH:DH + 1], 1.0)
        pt_pool = es_c.enter_context(tc.tile_pool(name="pt", bufs=6))
        att_sm = es_c.enter_context(tc.tile_pool(name="attsm", bufs=4))
        attnT_pool = es_c.enter_context(tc.tile_pool(name="attnT", bufs=2))
        ysb_pool = es_c.enter_context(tc.tile_pool(name="ysb", bufs=4))
        # PSUM: 8 banks total across these four pools
        ps_tr = es_c.enter_context(tc.tile_pool(name="ps_tr", bufs=1, space="PSUM"))
        ps_qkv = es_c.enter_context(tc.tile_pool(name="ps_qkv", bufs=2, space="PSUM"))
        ps_sc = es_c.enter_context(tc.tile_pool(name="ps_sc", bufs=3, space="PSUM"))
        ps_av = es_c.enter_context(tc.tile_pool(name="ps_av", bufs=2, space="PSUM"))

        y_part = [dram.tile([512, D], bf16, tag=f"y_part{g}",
                             name=f"y_part{g}") for g in range(NG)]
        y_red = [dram.tile([256, D], bf16, tag=f"y_red{g}",
                            name=f"y_red{g}") for g in range(NG)]

        def qk_slice(t, hh, cols):
            return t[DH * (hh % 2):DH * (hh % 2) + DH, hh // 2, cols]

        def ln1_chunk(ch):
            """LN1 stats + rsqrt + apply + transpose for one 512-token chunk.

            All rsqrts are emitted before the first exp, so the Act engine
            loads the sqrt table once and the exp table once.
            """
            for h2 in range(2):
                xt2 = x0_pool.tile([P, 2, D], bf16, tag="xt2")
                eng = nc.scalar if (2 * ch + h2) % 2 == 0 else nc.sync
                r0 = ch * 512 + h2 * 256
                eng.dma_start(xt2[:],
                              x_in[r0:r0 + 256, :]
                              .rearrange("(a p) d -> p a d", p=P))
                for i in range(2):
                    tt = 4 * ch + 2 * h2 + i
                    st6 = pa.tile([P, 2, 6], f32, tag="st6")
                    nc.vector.bn_stats(st6[:, 0, :], xt2[:, i, 0:512])
                    nc.vector.bn_stats(st6[:, 1, :], xt2[:, i, 512:1024])
                    nc.vector.bn_aggr(mv1[:, tt, :], st6[:])
                sl = slice(4 * ch + 2 * h2, 4 * ch + 2 * h2 + 2)
                nc.scalar.activation(rstd1[:, sl], mv1[:, sl, 1], AF.Sqrt,
                                     bias=eps_sb[:])
                nc.vector.reciprocal(rstd1[:, sl], rstd1[:, sl])
                nc.vector.tensor_scalar_mul(nmn1[:, sl], mv1[:, sl, 0], -1.0)
                for i in range(2):
                    tt = 4 * ch + 2 * h2 + i
                    xn = x0_pool.tile([P, D], bf16, tag="xn")
                    nc.vector.tensor_scalar(xn[:], xt2[:, i, :],
                                            nmn1[:, tt:tt + 1],
                                            rstd1[:, tt:tt + 1],
                                            ALU.add, ALU.mult)
                    ps = ps_tr.tile([P, 1024], bf16, tag="tr")
                    for ft in range(FT):
                        nc.tensor.transpose(ps[:, ft * P:(ft + 1) * P],
                                            xn[:, ft * P:(ft + 1) * P],
                                            idb_sb[:])
                    nc.vector.tensor_copy(
                        x_lnT[:, :, tt * P:(tt + 1) * P],
                        ps[:].rearrange("p (a b) -> p a b", a=8))

        def qkv_chunk(ch):
            c0 = ch * 512
            # K and Q (feature-major out)
            for wb, bias, dstT in [(w_mm["wk"], bk_sb, kT),
                                   (w_mm["wq"], bq_sb, qT)]:
                for ot in range(KFT):
                    ps = ps_qkv.tile([P, 512], f32, tag="mm")
                    if use_dr:
                        for tc2 in range(2):
                            for t in range(4):
                                nc.tensor.matmul(
                                    ps[:, tc2 * 256:(tc2 + 1) * 256],
                                    wb[:, 2 * t:2 * t + 2, ot * P:(ot + 1) * P],
                                    x_lnT[:, 2 * t:2 * t + 2,
                                          c0 + tc2 * 256:c0 + (tc2 + 1) * 256],
                                    start=(t == 0), stop=(t == 3), perf_mode=DR)
                    else:
                        for ft in range(FT):
                            nc.tensor.matmul(
                                ps[:], wb[:, ft, ot * P:(ot + 1) * P],
                                x_lnT[:, ft, c0:c0 + 512],
                                start=(ft == 0), stop=(ft == FT - 1))
                    nc.vector.tensor_scalar(
                        dstT[:, ot, c0:c0 + 512], ps[:],
                        invws, bias[:, ot:ot + 1], ALU.mult, ALU.add)
            # V (token-major out)
            for tt in range(4 * ch, 4 * ch + 4):
                ps = ps_qkv.tile([P, 512], f32, tag="mm")
                if use_dr:
                    for vc in range(2):
                        for t in range(4):
                            nc.tensor.matmul(
                                ps[:, vc * 256:(vc + 1) * 256],
                                x_lnT[:, 2 * t:2 * t + 2, tt * P:(tt + 1) * P],
                                w_mm["wv"][:, 2 * t:2 * t + 2,
                                           vc * 256:(vc + 1) * 256],
                                start=(t == 0), stop=(t == 3), perf_mode=DR)
                else:
                    for ft in range(FT):
                        nc.tensor.matmul(
                            ps[:], x_lnT[:, ft, tt * P:(tt + 1) * P],
                            w_mm["wv"][:, ft, :],
                            start=(ft == 0), stop=(ft == FT - 1))
                nc.vector.scalar_tensor_tensor(
                    v_sb[:, tt, :, 0:DH],
                    ps[:].rearrange("p (h d) -> p h d", d=DH), invws,
                    bv_r[:].rearrange("p (h d) -> p h d", d=DH),
                    ALU.mult, ALU.add)

        def attention_group(g):
            def normalize(hh, av):
                # 1/l per query; broadcast over partitions via PE outer prod
                recip = att_sm.tile([1, 512], bf16, tag="recip")
                with nc.allow_low_precision(reason="softmax 1/l in bf16"):
                    nc.vector.reciprocal(recip[:], av[DH:DH + 1, :])
                ps_rp = ps_sc.tile([P, 512], f32, tag="sc")
                nc.tensor.matmul(ps_rp[0:DH, :], ones64[:], recip[:],
                                 start=True, stop=True)
                rep = att_sm.tile([DH, 512], f32, tag="rep_sb")
                nc.vector.tensor_copy(rep[:], ps_rp[0:DH, :])
                nc.vector.tensor_mul(
                    attnT_g[DH * (hh % 2):DH * (hh % 2) + DH, hh // 2, :],
                    av[0:DH, :], rep[:])

            prev = None
            for hh in range(HPC):
                av = ps_av.tile([P, 512], f32, tag="av")
                # j2 pairs: both score matmuls + exps, then both AV chains --
                # the second score matmul hides the first exp's latency.
                for p2 in range(0, 4 * g + 4, 2):
                    pts = []
                    for j2 in (p2, p2 + 1):
                        p0 = max(0, j2 - 4 * g) * P
                        ps_s = ps_sc.tile([P, 512], f32, tag="sc")
                        nc.tensor.matmul(
                            ps_s[:, p0:512],
                            qk_slice(kT, hh, slice(j2 * P, (j2 + 1) * P)),
                            qk_slice(qT, hh,
                                     slice(g * 512 + p0, (g + 1) * 512)),
                            start=True, stop=True)
                        pT = pt_pool.tile([P, 512], bf16, tag="pT")
                        nc.scalar.activation(pT[:, p0:512], ps_s[:, p0:512],
                                             AF.Exp, scale=scale)
                        if j2 >= 4 * g:
                            nc.gpsimd.tensor_mul(pT[:, p0:p0 + P],
                                                 pT[:, p0:p0 + P], tri_sb[:])
                        pts.append(pT)
                    for j2, pT in zip((p2, p2 + 1), pts):
                        if j2 == 0:
                            nc.tensor.matmul(
                                av[0:DH + 1, 0:512], v_sb[:, 0, hh, :],
                                pT[:, 0:512], start=True, stop=False)
                        else:
                            q0 = max(0, j2 - 4 * g)
                            if q0 == 0:
                                nc.tensor.matmul(
                                    av[0:DH + 1, 0:512], v_sb[:, j2, hh, :],
                                    pT[:, 0:512], start=False,
                                    stop=(j2 == 4 * g + 3))
                            else:
                                for qb in range(q0, 4):
                                    nc.tensor.matmul(
                                        av[0:DH + 1, qb * P:(qb + 1) * P],
                                        v_sb[:, j2, hh, :],
                                        pT[:, qb * P:(qb + 1) * P],
                                        start=False,
                                        stop=(j2 == 4 * g + 3 and qb == 3))
                # defer normalize one head so its DVE/PE chain overlaps the
                # next head's score loop
                if prev is not None:
                    normalize(*prev)
                prev = (hh, av)
            normalize(*prev)

        def proj_group(g):
            for tt2 in range(4):
                ysb = ysb_pool.tile([P, D], bf16, tag="ysb")
                for oc in range(2):
                    ps = ps_av.tile([P, 512], f32, tag="av")
                    for ftk in range(KFT):
                        nc.tensor.matmul(
                            ps[:],
                            attnT_g[:, ftk, tt2 * P:(tt2 + 1) * P],
                            pw_mm[:, ftk, oc * 512:(oc + 1) * 512],
                            start=(ftk == 0), stop=(ftk == KFT - 1))
                    nc.vector.tensor_copy(ysb[:, oc * 512:(oc + 1) * 512],
                                          ps[:])
                nc.sync.dma_start(y_part[g][tt2 * P:(tt2 + 1) * P, :], ysb[:])
            nc.gpsimd.collective_compute(
                "ReduceScatter", ALU.add, replica_groups=groups,
                ins=[y_part[g].opt()], outs=[y_red[g].opt()])

        # ======== interleaved pipeline ========
        # each attention group is emitted right after its own QKV chunk so
        # the Act exp pipeline starts as early as possible
        x0_pool = es_0.enter_context(tc.tile_pool(name="x0", bufs=2))
        pa = es_0.enter_context(tc.tile_pool(name="pa", bufs=3))
        ln1_chunk(0)
        load_qkv_weights()
        ln1_chunk(1)
        qkv_chunk(0)
        # mlp weights/biases: needed only in the post phase; queued behind
        # everything the attention span needs
        if use_dr:
            nc.scalar.dma_start(fc1w_sb[:], fc1w_in[:])
        nc.scalar.dma_start(fc1b_sb[:], fc1b_in[:])
        nc.scalar.dma_start(fc2b_r[:], fc2b_in[None, :].to_broadcast((P, D)))
        attnT_g = attnT_pool.tile([P, KFT, 512], bf16, tag="attnT")
        attention_group(0)
        proj_group(0)
        ln1_chunk(2)
        ln1_chunk(3)
        es_0.close()
        for g in range(1, NG):
            qkv_chunk(g)
            attnT_g = attnT_pool.tile([P, KFT, 512], bf16, tag="attnT")
            attention_group(g)
            proj_group(g)

        es_c.close()
        es_a.close()

        # fc2 weights: loaded into the space attention just freed; the DMA
        # overlaps the attention/proj drain.
        wres2 = es.enter_context(tc.tile_pool(name="wres2", bufs=1))
        fc2w_sb = wres2.tile([P, MT, D], mmdt)
        nc.scalar.dma_start(fc2w_sb[:], fc2w_in[:])

        # ======== post-attention: x1/LN2/FC1/FC2 per 512-token chunk ========
        post = es.enter_context(tc.tile_pool(name="post", bufs=2))
        x1pool = es.enter_context(tc.tile_pool(name="x1p", bufs=8))
        x1ln = es.enter_context(tc.tile_pool(name="x1ln", bufs=1))
        x1_lnT = x1ln.tile([P, FT, MY], mmdt)
        hT_pool = es.enter_context(
            tc.tile_pool(name="hT", bufs=2 if use_dr else 1))
        w1pool = None if use_dr else es.enter_context(
            tc.tile_pool(name="w1pool", bufs=4))
        out_pool = es.enter_context(tc.tile_pool(name="outp", bufs=2))
        ps_fc1 = es.enter_context(tc.tile_pool(name="ps_fc1", bufs=2, space="PSUM"))
        ps_fc2 = es.enter_context(tc.tile_pool(name="ps_fc2", bufs=2, space="PSUM"))
        ps_tr2 = es.enter_context(tc.tile_pool(name="ps_tr2", bufs=2, space="PSUM"))

        # x1 = y_red + (x_res + pb)  [f32] stays resident in SBUF through
        # the k-chunk: LN2 apply + the FC2 residual read it directly.
        for k in range(2):
            x1ts_k = {}
            for h2 in range(2):
                tts = (4 * k + 2 * h2, 4 * k + 2 * h2 + 1)
                for tt in tts:
                    g, half = tt // 2, tt % 2
                    yr = post.tile([P, D], bf16, tag="yr")
                    nc.sync.dma_start(
                        yr[:], y_red[g][half * P:(half + 1) * P, :])
                    xr = post.tile([P, D], f32, tag="xr")
                    nc.gpsimd.dma_start(xr[:],
                                        xres_in[tt * P:(tt + 1) * P, :])
                    x1t = x1pool.tile([P, D], f32, tag="x1t")
                    nc.vector.tensor_add(x1t[:], xr[:], yr[:])
                    st6 = post.tile([P, 2, 6], f32, tag="st6b")
                    nc.vector.bn_stats(st6[:, 0, :], x1t[:, 0:512])
                    nc.vector.bn_stats(st6[:, 1, :], x1t[:, 512:1024])
                    nc.vector.bn_aggr(mv2[:, tt % 8, :], st6[:])
                    x1ts_k[tt] = x1t
                sl = slice(tts[0] % 8, tts[0] % 8 + 2)
                nc.scalar.activation(rstd2[:, sl], mv2[:, sl, 1], AF.Sqrt,
                                     bias=eps_sb[:])
                nc.vector.reciprocal(rstd2[:, sl], rstd2[:, sl])
                nc.vector.tensor_scalar_mul(nmn2[:, sl], mv2[:, sl, 0], -1.0)
                for tt in tts:
                    xn2 = post.tile([P, D], bf16, tag="xn2")
                    nc.vector.tensor_scalar(xn2[:], x1ts_k[tt][:],
                                            nmn2[:, tt % 8:tt % 8 + 1],
                                            rstd2[:, tt % 8:tt % 8 + 1],
                                            ALU.add, ALU.mult)
                    ps = ps_tr2.tile([P, 1024], bf16, tag="tr2")
                    for ft in range(FT):
                        nc.tensor.transpose(ps[:, ft * P:(ft + 1) * P],
                                            xn2[:, ft * P:(ft + 1) * P],
                                            idb_sb[:])
                    nc.vector.tensor_copy(
                        x1_lnT[:, :, tt * P:(tt + 1) * P],
                        ps[:].rearrange("p (a b) -> p a b", a=8))

            # FC1 + gelu -> hT (feature-major)
            c0 = k * 512
            hT = hT_pool.tile([P, MT, 512], mmdt, tag="hT")
            for ot in range(MT):
                if use_dr:
                    wsl = fc1w_sb[:, :, ot * P:(ot + 1) * P]
                else:
                    if ot % 2 == 0:
                        w1stg = w1pool.tile([P, FT, 2 * P], bf16, tag="w1stg")
                        eng1 = nc.sync if ot % 4 == 0 else nc.gpsimd
                        eng1.dma_start(w1stg[:],
                                       fc1w_in[:, :, ot * P:(ot + 2) * P])
                    wsl = w1stg[:, :, (ot % 2) * P:(ot % 2 + 1) * P]
                ps = ps_fc1.tile([P, 512], f32, tag="mm1")
                if use_dr:
                    for tc2 in range(2):
                        for t in range(4):
                            nc.tensor.matmul(
                                ps[:, tc2 * 256:(tc2 + 1) * 256],
                                wsl[:, 2 * t:2 * t + 2, :],
                                x1_lnT[:, 2 * t:2 * t + 2,
                                       c0 + tc2 * 256:c0 + (tc2 + 1) * 256],
                                start=(t == 0), stop=(t == 3), perf_mode=DR)
                else:
                    for ft in range(FT):
                        nc.tensor.matmul(
                            ps[:], wsl[:, ft, :],
                            x1_lnT[:, ft, c0:c0 + 512],
                            start=(ft == 0), stop=(ft == FT - 1))
                nc.scalar.activation(hT[:, ot, :], ps[:], AF.Gelu,
                                     bias=fc1b_sb[:, ot:ot + 1], scale=invws)

            # FC2 (token-major out) + residual + store
            for tt in range(4 * k, 4 * k + 4):
                ot_sb = out_pool.tile([P, D], f32, tag="x1o")
                x1t = x1ts_k[tt]
                tloc = (tt - 4 * k) * P
                if use_dr:
                    for oc in range(4):
                        ps = ps_fc2.tile([P, 256], f32, tag="mm2")
                        for t in range(MT // 2):
                            nc.tensor.matmul(
                                ps[:], hT[:, 2 * t:2 * t + 2, tloc:tloc + P],
                                fc2w_sb[:, 2 * t:2 * t + 2,
                                        oc * 256:(oc + 1) * 256],
                                start=(t == 0), stop=(t == MT // 2 - 1),
                                perf_mode=DR)
                        nc.vector.scalar_tensor_tensor(
                            ot_sb[:, oc * 256:(oc + 1) * 256], ps[:], invws,
                            x1t[:, oc * 256:(oc + 1) * 256],
                            ALU.mult, ALU.add)
                else:
                    for oc in range(2):
                        ps = ps_fc2.tile([P, 512], f32, tag="mm2")
                        for kt in range(MT):
                            nc.tensor.matmul(
                                ps[:], hT[:, kt, tloc:tloc + P],
                                fc2w_sb[:, kt, oc * 512:(oc + 1) * 512],
                                start=(kt == 0), stop=(kt == MT - 1))
                        nc.vector.scalar_tensor_tensor(
                            ot_sb[:, oc * 512:(oc + 1) * 512], ps[:], invws,
                            x1t[:, oc * 512:(oc + 1) * 512],
                            ALU.mult, ALU.add)
                nc.vector.tensor_add(ot_sb[:], ot_sb[:], fc2b_r[:])
                nc.scalar.dma_start(out_dram[tt * P:(tt + 1) * P, :], ot_sb[:])
    return nc


# ---------------- host side ----------------

DIM = 1024
HEADS = 16
HEAD_DIM = DIM // HEADS
MLP_DIM = 4 * DIM
SEQ = 2048
BATCH = 4
N_CORES = 8

_nc_cache = {}


def _get_nc(mm="fp8", num_devices=N_CORES):
    key = (mm, num_devices)
    if key not in _nc_cache:
        nc = build_nc(S=SEQ, D=DIM, H=HEADS, MLP=MLP_DIM,
                      num_devices=num_devices, mm=mm)
        nc.compile()
        _nc_cache[key] = nc
    return _nc_cache[key]


def _tile_w(w, np_dt):
    """[D_in, N] -> [128, D_in//128, N] device layout (ki ko o)."""
    d_in, n = w.shape
    return np.ascontiguousarray(
        w.reshape(d_in // P, P, n).transpose(1, 0, 2).astype(np_dt))


def _tile_b(b, np_dt=np.float32):
    """[N] -> [128, N//128] device layout (p o)."""
    n = b.shape[0]
    return np.ascontiguousarray(b.reshape(n // P, P).T.astype(np_dt))


def make_in_maps(x, ln1_w, ln1_b, qkv_w, qkv_b, proj_w, proj_b,
                 ln2_w, ln2_b, fc1_w, fc1_b, fc2_w, fc2_b,
                 mm="fp8", S=SEQ, D=DIM, H=HEADS, n_cores=N_CORES):
    DHh = D // H
    HPC = H // 2
    ws = WS if mm == "fp8" else 1.0
    mmdt_np = ml_dtypes.float8_e4m3 if mm == "fp8" else ml_dtypes.bfloat16
    f = np.float32
    ln1_w = np.asarray(ln1_w, f); ln1_b = np.asarray(ln1_b, f)
    ln2_w = np.asarray(ln2_w, f); ln2_b = np.asarray(ln2_b, f)
    qkv_w = np.asarray(qkv_w, f); qkv_b = np.asarray(qkv_b, f)
    proj_w = np.asarray(proj_w, f); proj_b = np.asarray(proj_b, f)
    fc1_w = np.asarray(fc1_w, f); fc1_b = np.asarray(fc1_b, f)
    fc2_w = np.asarray(fc2_w, f); fc2_b = np.asarray(fc2_b, f)
    x = np.asarray(x, f)

    # fold LN affine into the downstream linear (exact):
    #   LN(x) @ W + b == xhat @ (w[:,None]*W) + (ln_b @ W + b)
    qkv_w_f = (ln1_w[:, None] * qkv_w) * ws
    qkv_b_f = ln1_b @ qkv_w + qkv_b
    fc1_w_f = (ln2_w[:, None] * fc1_w) * ws
    fc1_b_f = ln2_b @ fc1_w + fc1_b
    fc2_w_f = fc2_w * ws

    qkv_w4 = np.ascontiguousarray(qkv_w_f.reshape(D, 3, H, DHh))
    qkv_b3 = np.ascontiguousarray(qkv_b_f.reshape(3, H, DHh))
    proj_w3 = np.ascontiguousarray(proj_w.reshape(H, DHh, D))
    tri = np.triu(np.ones((P, P))).astype(ml_dtypes.bfloat16)
    identb = np.eye(P).astype(ml_dtypes.bfloat16)

    # owned strips: member m owns [512g+256m, 512g+256m+256) for g=0..3
    common = {
        "fc1w": _tile_w(fc1_w_f, mmdt_np),
        "fc1b": _tile_b(fc1_b_f),
        "fc2w": _tile_w(fc2_w_f, mmdt_np),
        "fc2b": np.ascontiguousarray(fc2_b),
        "tri": tri, "identb": identb,
    }
    in_maps = []
    for c in range(n_cores):
        b, m = c // 2, c % 2
        heads = slice(m * HPC, (m + 1) * HPC)
        im = dict(common)
        im["x"] = np.ascontiguousarray(x[b].astype(ml_dtypes.bfloat16))
        strips = [x[b, 512 * g + 256 * m: 512 * g + 256 * m + 256] + proj_b
                  for g in range(4)]
        im["x_res"] = np.ascontiguousarray(np.concatenate(strips, axis=0), f)
        im["wq"] = _tile_w(qkv_w4[:, 0, heads].reshape(D, -1), mmdt_np)
        im["wk"] = _tile_w(qkv_w4[:, 1, heads].reshape(D, -1), mmdt_np)
        im["wv"] = _tile_w(qkv_w4[:, 2, heads].reshape(D, -1), mmdt_np)
        im["bq"] = _tile_b(qkv_b3[0, heads].reshape(-1))
        im["bk"] = _tile_b(qkv_b3[1, heads].reshape(-1))
        im["bv"] = np.ascontiguousarray(qkv_b3[2, heads].reshape(-1))
        im["pw"] = _tile_w(proj_w3[heads].reshape(-1, D), ml_dtypes.bfloat16)
        in_maps.append(im)
    return in_maps


def assemble_out(results, S=SEQ, D=DIM, B=BATCH):
    full = np.empty((B, S, D), dtype=np.float32)
    for c in range(len(results)):
        b, m = c // 2, c % 2
        o = results[c]["out"]
        for g in range(4):
            full[b, 512 * g + 256 * m: 512 * g + 256 * m + 256] = \
                o[256 * g: 256 * g + 256]
    return full


def kernel(x, ln1_w, ln1_b, qkv_w, qkv_b, proj_w, proj_b,
           ln2_w, ln2_b, fc1_w, fc1_b, fc2_w, fc2_b):
    import os
    from concourse.bass_utils import run_bass_kernel_spmd
    mm = os.environ.get("MM_MODE", "fp8")
    nc = _get_nc(mm=mm)
    in_maps = make_in_maps(np.asarray(x), ln1_w, ln1_b, qkv_w, qkv_b,
                           proj_w, proj_b, ln2_w, ln2_b,
                           fc1_w, fc1_b, fc2_w, fc2_b, mm=mm)
    res = run_bass_kernel_spmd(nc, in_maps, list(range(N_CORES)), trace=False)
    return assemble_out(res.results)
